# revision 1
# baseline (speedup 1.0000x reference)
"""Bottleneck-MHSA fused kernel for 8 Trainium2 NeuronCores.

Sharding: core c = 2*b + s handles batch b; attention queries are split in
half between the two cores of a pair. Each core computes conv1 + BN1 + qkv
for its whole batch (redundantly with its pair partner), then attention for
all 4 heads over its query half, then BN2 + W3 + BN3 + residual for its
query half. Cross-core traffic is only the three tiny BN statistics
AllGathers.

Matmuls run in float32r (full-rate fp32, ~1e-4 rel err). Host-side prep:
weight transposes, rel_h+rel_w+rel_d fusion, batch token rotation so every
core's query half is columns 0:1372 of its own input (keeps the device
program identical across cores; attention is permutation-invariant over
keys). ScalarE runs exp only (all relu/rsqrt on DVE) so the activation
table loads once.
"""
import numpy as np

HEADS = 4
DH = 64
C = 256          # PLANES
CIN = 1024       # IN_PLANES
N = 2744         # tokens per batch
NQ = N // 2      # query half per core
B = 4
EPS = 1e-5
N_CORES = 8
CNT12 = 8 * N    # BN1 effective count (pairs double-count; mean/var exact)
CNT2 = 8 * NQ    # BN2/BN3 count (distinct shards)


def _chunks(total, n):
    # even sizes (fp32r matmul requires an even moving free dim)
    assert total % 2 == 0
    half = total // 2
    sizes = [2 * (half // n + (1 if i < half % n else 0)) for i in range(n)]
    out, off = [], 0
    for s in sizes:
        out.append((off, s))
        off += s
    return out


CH6 = _chunks(N, 6)     # key/token chunks (456/458 wide, all >=256 for f32r)
CH3 = _chunks(NQ, 3)    # query chunks
MT22 = [(t * 128, min(128, N - t * 128)) for t in range((N + 127) // 128)]


def build_program():
    from concourse import bacc, mybir, tile

    F32 = mybir.dt.float32
    F32R = mybir.dt.float32r
    I32 = mybir.dt.int32

    nc = bacc.Bacc("TRN2", target_bir_lowering=False, debug=False,
                   num_devices=N_CORES)

    # ---- I/O ----
    io = {}
    io["X"] = nc.dram_tensor("X", [CIN, N], F32R, kind="ExternalInput").ap()
    io["W1T"] = nc.dram_tensor("W1T", [CIN, C], F32R, kind="ExternalInput").ap()
    io["WQT"] = nc.dram_tensor("WQT", [C, C], F32R, kind="ExternalInput").ap()
    io["WVT"] = nc.dram_tensor("WVT", [C, C], F32R, kind="ExternalInput").ap()
    io["W3T"] = nc.dram_tensor("W3T", [C, CIN], F32R, kind="ExternalInput").ap()
    io["WKQ"] = nc.dram_tensor("WKQ", [HEADS, C, 128], F32R, kind="ExternalInput").ap()
    io["REL"] = nc.dram_tensor("REL", [HEADS, DH, NQ], F32R, kind="ExternalInput").ap()
    io["BKQ"] = nc.dram_tensor("BKQ", [HEADS, 128], F32, kind="ExternalInput").ap()
    io["BQ"] = nc.dram_tensor("BQ", [HEADS, DH], F32, kind="ExternalInput").ap()
    io["BVR"] = nc.dram_tensor("BVR", [1, C], F32R, kind="ExternalInput").ap()
    io["GB1"] = nc.dram_tensor("GB1", [2, C], F32, kind="ExternalInput").ap()
    io["GB2"] = nc.dram_tensor("GB2", [2, C], F32, kind="ExternalInput").ap()
    io["GB3"] = nc.dram_tensor("GB3", [2, CIN], F32, kind="ExternalInput").ap()
    io["OUT"] = nc.dram_tensor("OUT", [CIN, NQ], F32, kind="ExternalOutput").ap()

    with tile.TileContext(nc) as tc:
        _emit(nc, tc, mybir, F32, F32R, I32, io)

    nc.compile()
    from concourse.bass_interp import get_hw_module
    nc.m = get_hw_module(nc.m)
    return nc


def _emit(nc, tc, mybir, F32, F32R, I32, io):
    import contextlib

    AX = mybir.AluOpType
    AF = mybir.ActivationFunctionType
    X_AXIS = mybir.AxisListType.X

    Xd, W1T, WQT, WVT, W3T = io["X"], io["W1T"], io["WQT"], io["WVT"], io["W3T"]
    WKQ, RELd, BKQ, BQd, BVR = io["WKQ"], io["REL"], io["BKQ"], io["BQ"], io["BVR"]
    GB1, GB2, GB3, OUTd = io["GB1"], io["GB2"], io["GB3"], io["OUT"]

    def stats_collective(src_sbuf, width, out_gst, tag):
        """AllGather [P, width] partials over all 8 cores; out_gst[P, width] =
        sum over cores."""
        p = src_sbuf.shape[0]
        cin = dpool.tile([p, width], F32, name=f"ccin_{tag}")
        cout = dpool.tile([N_CORES, p, width], F32, addr_space="Shared",
                          name=f"ccout_{tag}")
        nc.sync.dma_start(cin[:], src_sbuf[:])
        nc.gpsimd.collective_compute(
            "AllGather", AX.bypass,
            replica_groups=[list(range(N_CORES))],
            ins=[cin.opt()], outs=[cout.opt()],
        )
        gath = wpool.tile([p, width, N_CORES], F32, name=f"gath_{tag}")
        nc.sync.dma_start(gath[:], cout[:].rearrange("g p c -> p c g"))
        nc.vector.reduce_sum(out_gst[:], gath[:], X_AXIS)

    def rsqrt_newton(y, x, tag):
        """y = 1/sqrt(x) on DVE only (bit-trick seed + 2 Newton steps)."""
        p, w = x.shape[0], x.shape[1]
        xi = x[:].bitcast(I32)
        t1 = wpool.tile([p, w], I32, name=f"rsq_t1_{tag}")
        nc.vector.tensor_scalar(t1[:], xi, 1, None, AX.arith_shift_right)
        yi = y[:].bitcast(I32)
        nc.vector.tensor_scalar(yi, t1[:], -1, 0x5f3759df, AX.mult, AX.add)
        h = wpool.tile([p, w], F32, name=f"rsq_h_{tag}")
        for _ in range(2):
            nc.vector.tensor_tensor(h[:], y[:], y[:], AX.mult)
            nc.vector.tensor_tensor(h[:], x[:], h[:], AX.mult)
            nc.vector.tensor_scalar(h[:], h[:], -0.5, 1.5, AX.mult, AX.add)
            nc.vector.tensor_tensor(y[:], y[:], h[:], AX.mult)

    def bn_coeffs(tot, gt, bt, cnt, w, sc, cc, tag):
        """tot [P, 2w] = [sums | sumsqs] -> scale sc [P, w], bias cc [P, w]."""
        p = tot.shape[0]
        mean = wpool.tile([p, w], F32, name=f"mean_{tag}")
        var = wpool.tile([p, w], F32, name=f"var_{tag}")
        nc.vector.tensor_scalar_mul(mean[:], tot[:, 0:w], 1.0 / cnt)
        nc.vector.tensor_scalar_mul(var[:], tot[:, w:2 * w], 1.0 / cnt)
        m2 = wpool.tile([p, w], F32, name=f"m2_{tag}")
        nc.vector.tensor_tensor(m2[:], mean[:], mean[:], AX.mult)
        nc.vector.tensor_tensor(var[:], var[:], m2[:], AX.subtract)
        nc.vector.tensor_scalar_add(var[:], var[:], EPS)
        sd = wpool.tile([p, w], F32, name=f"sd_{tag}")
        nc.scalar.activation(sd[:], var[:], AF.Sqrt)
        rstd = wpool.tile([p, w], F32, name=f"rstd_{tag}")
        nc.vector.reciprocal(rstd[:], sd[:])
        nc.vector.tensor_tensor(sc[:], gt[:], rstd[:], AX.mult)
        tmp = wpool.tile([p, w], F32, name=f"tmpc_{tag}")
        nc.vector.tensor_tensor(tmp[:], sc[:], mean[:], AX.mult)
        nc.vector.tensor_tensor(cc[:], bt[:], tmp[:], AX.subtract)

    with contextlib.ExitStack() as top:
        wpool = top.enter_context(tc.tile_pool(name="wpool", bufs=1))
        dpool = top.enter_context(tc.tile_pool(name="dpool", bufs=1, space="DRAM"))

        # ---- weights / constants ----
        w1t = []
        for k in range(8):
            t = wpool.tile([128, C], F32R, name=f"w1t{k}")
            nc.scalar.dma_start(t[:], W1T[k * 128:(k + 1) * 128, :])
            w1t.append(t)
        wqt, wvt = [], []
        for srcw, dst, nm in ((WQT, wqt, "wq"), (WVT, wvt, "wv")):
            for k in range(2):
                t = wpool.tile([128, C], F32R, name=f"{nm}{k}")
                nc.scalar.dma_start(t[:], srcw[k * 128:(k + 1) * 128, :])
                dst.append(t)
        wkqt = []
        for h in range(HEADS):
            row = []
            for k in range(2):
                t = wpool.tile([128, 128], F32R, name=f"wkq{h}_{k}")
                nc.scalar.dma_start(t[:], WKQ[h][k * 128:(k + 1) * 128, :])
                row.append(t)
            wkqt.append(row)
        w3t = []
        for k in range(2):
            t = wpool.tile([128, CIN], F32R, name=f"w3t{k}")
            nc.scalar.dma_start(t[:], W3T[k * 128:(k + 1) * 128, :])
            w3t.append(t)
        bvrow = wpool.tile([1, C], F32R, name="bvrow")
        nc.scalar.dma_start(bvrow[:], BVR[:])

        bkqt = wpool.tile([128, HEADS], F32, name="bkqt")
        nc.scalar.dma_start(bkqt[:], BKQ[:].rearrange("h p -> p h"))
        bqt = wpool.tile([DH, HEADS], F32, name="bqt")
        nc.scalar.dma_start(bqt[:], BQd[:].rearrange("h d -> d h"))
        g1t = wpool.tile([128, 2], F32, name="g1t")
        b1t = wpool.tile([128, 2], F32, name="b1t")
        nc.scalar.dma_start(g1t[:], GB1[0].rearrange("(m p) -> p m", p=128))
        nc.scalar.dma_start(b1t[:], GB1[1].rearrange("(m p) -> p m", p=128))
        g2t = wpool.tile([DH, HEADS], F32, name="g2t")
        b2t = wpool.tile([DH, HEADS], F32, name="b2t")
        nc.scalar.dma_start(g2t[:], GB2[0].rearrange("(h d) -> d h", d=DH))
        nc.scalar.dma_start(b2t[:], GB2[1].rearrange("(h d) -> d h", d=DH))
        g3t = wpool.tile([128, 8], F32, name="g3t")
        b3t = wpool.tile([128, 8], F32, name="b3t")
        nc.scalar.dma_start(g3t[:], GB3[0].rearrange("(m p) -> p m", p=128))
        nc.scalar.dma_start(b3t[:], GB3[1].rearrange("(m p) -> p m", p=128))

        # f32r constants (memset to f32r is rejected at codegen; go via f32)
        onespad = wpool.tile([128, 128], F32, name="onespad")
        nc.vector.memset(onespad[:], 1.0)
        onesr = wpool.tile([128, DH], F32R, name="onesr")
        nc.vector.tensor_copy(onesr[:], onespad[:, 0:DH])
        onesrow = wpool.tile([1, 128], F32R, name="onesrow")
        nc.vector.tensor_copy(onesrow[:], onespad[0:1, :])
        onescol = wpool.tile([128, 1], F32, name="onescol")
        nc.vector.memset(onescol[:], 1.0)

        # stats accumulators
        S1 = wpool.tile([128, 12], F32, name="S1")   # conv1 sums   (mt*6+ci)
        Q1 = wpool.tile([128, 12], F32, name="Q1")   # conv1 sumsqs
        S2 = wpool.tile([DH, 16], F32, name="S2")    # attn sums    (h*4+ci)
        nc.vector.memset(S2[:], 0.0)
        Q2 = wpool.tile([DH, 16], F32, name="Q2")
        nc.vector.memset(Q2[:], 0.0)
        S3 = wpool.tile([128, 24], F32, name="S3")   # W3 sums      (mt*3+ci)
        Q3 = wpool.tile([128, 24], F32, name="Q3")

        OUT2 = [wpool.tile([128, NQ], F32R, name=f"out2_{m}") for m in range(2)]

        with contextlib.ExitStack() as ph_a:
            qpool = ph_a.enter_context(tc.tile_pool(name="qpool", bufs=1))
            KHAT = [qpool.tile([128, N], F32R, name=f"khat{h}") for h in range(HEADS)]
            QHAT = [qpool.tile([128, NQ], F32R, name=f"qhat{h}") for h in range(HEADS)]
            VTON = [qpool.tile([128, 22 * 65], F32R, name=f"vton{h}") for h in range(HEADS)]
            for h in range(HEADS):
                nc.scalar.dma_start(QHAT[h][DH:128, :], RELd[h])

            with contextlib.ExitStack() as ph1:
                y1pool = ph1.enter_context(tc.tile_pool(name="y1pool", bufs=1))
                Y1 = [y1pool.tile([128, N], F32R, name=f"y1_{m}") for m in range(2)]

                # ---- phase 1: conv1 (y1 = W1 @ x), stats partials ----
                with tc.tile_pool(name="xpool", bufs=2) as xpool, \
                     tc.tile_pool(name="psum1", bufs=3, space="PSUM") as psum1:
                    for ci, (off, sz) in enumerate(CH6):
                        xts = []
                        for k in range(8):
                            t = xpool.tile([128, sz], F32R, name=f"xc{k}",
                                           tag=f"xc{k}")
                            eng = nc.sync if k % 2 == 0 else nc.gpsimd
                            eng.dma_start(t[:], Xd[k * 128:(k + 1) * 128,
                                                   off:off + sz])
                            xts.append(t)
                        for mt in range(2):
                            ps = psum1.tile([128, sz], F32, name="pconv", tag="pconv")
                            for k in range(8):
                                nc.tensor.matmul(ps[:], w1t[k][:, mt * 128:(mt + 1) * 128],
                                                 xts[k][:], start=(k == 0), stop=(k == 7))
                            idx = mt * 6 + ci
                            nc.vector.tensor_copy(Y1[mt][:, off:off + sz], ps[:])
                            nc.vector.reduce_sum(S1[:, idx:idx + 1], ps[:], X_AXIS)
                            sq = xpool.tile([128, sz], F32, name="sqs", tag="sqs")
                            nc.vector.tensor_tensor(sq[:], Y1[mt][:, off:off + sz],
                                                    Y1[mt][:, off:off + sz], AX.mult)
                            nc.vector.reduce_sum(Q1[:, idx:idx + 1], sq[:], X_AXIS)

                # ---- phase 1b: BN1 stats collective + coeffs ----
                s1sum = wpool.tile([128, 2], F32, name="s1sum")
                q1sum = wpool.tile([128, 2], F32, name="q1sum")
                nc.vector.reduce_sum(s1sum[:], S1[:].rearrange("p (m c) -> p m c", c=6), X_AXIS)
                nc.vector.reduce_sum(q1sum[:], Q1[:].rearrange("p (m c) -> p m c", c=6), X_AXIS)
                st1 = wpool.tile([128, 4], F32, name="st1")
                nc.vector.tensor_copy(st1[:, 0:2], s1sum[:])
                nc.vector.tensor_copy(st1[:, 2:4], q1sum[:])
                tot1 = wpool.tile([128, 4], F32, name="tot1")
                stats_collective(st1, 4, tot1, "bn1")
                s1c = wpool.tile([128, 2], F32, name="s1c")
                c1c = wpool.tile([128, 2], F32, name="c1c")
                bn_coeffs(tot1, g1t, b1t, CNT12, 2, s1c, c1c, "bn1")

                # ---- phase 2: out1 = relu(s*y1 + c), in place, DVE only ----
                OUT1 = [Y1[m][:] for m in range(2)]
                for mt in range(2):
                    for (off, sz) in CH6:
                        nc.vector.tensor_scalar(OUT1[mt][:, off:off + sz],
                                                Y1[mt][:, off:off + sz],
                                                s1c[:, mt:mt + 1], c1c[:, mt:mt + 1],
                                                AX.mult, AX.add)
                        nc.vector.tensor_scalar(OUT1[mt][:, off:off + sz],
                                                OUT1[mt][:, off:off + sz],
                                                0.0, None, AX.max)

                # ---- phase 3a: vT = out1^T @ WvT (+bv via ones-row), into VTON ----
                with tc.tile_pool(name="psum3a", bufs=3, space="PSUM") as psum3a:
                    for t, (mo, msz) in enumerate(MT22):
                        ps = psum3a.tile([128, C], F32, name="pvt", tag="pvt")
                        nc.tensor.matmul(ps[0:msz, :], onesrow[:, 0:msz], bvrow[:],
                                         start=True, stop=False)
                        for k in range(2):
                            nc.tensor.matmul(ps[0:msz, :], OUT1[k][:, mo:mo + msz],
                                             wvt[k][:], start=False, stop=(k == 1))
                        for h in range(HEADS):
                            nc.vector.tensor_copy(
                                VTON[h][0:msz, 65 * t:65 * t + DH],
                                ps[0:msz, h * DH:(h + 1) * DH])
                            nc.vector.tensor_scalar(
                                VTON[h][0:msz, 65 * t + DH:65 * t + 65],
                                onescol[0:msz, :], 0.0, None, AX.add)

                # ---- phase 3b: KHAT = [k; q] (packed weights), QHAT q-half ----
                with tc.tile_pool(name="psum3b", bufs=3, space="PSUM") as psum3b:
                    for h in range(HEADS):
                        hs = h * DH
                        for (off, sz) in CH6:
                            ps = psum3b.tile([128, sz], F32, name="pkh", tag="pkh")
                            for k in range(2):
                                nc.tensor.matmul(ps[:], wkqt[h][k][:],
                                                 OUT1[k][:, off:off + sz],
                                                 start=(k == 0), stop=(k == 1))
                            nc.vector.tensor_scalar(KHAT[h][:, off:off + sz], ps[:],
                                                    bkqt[:, h:h + 1], None, AX.add)
                        for (off, sz) in CH3:
                            pq = psum3b.tile([DH, sz], F32, name="pqh", tag="pqh")
                            for k in range(2):
                                nc.tensor.matmul(pq[:], wqt[k][:, hs:hs + DH],
                                                 OUT1[k][:, off:off + sz],
                                                 start=(k == 0), stop=(k == 1))
                            nc.vector.tensor_scalar(QHAT[h][0:DH, off:off + sz], pq[:],
                                                    bqt[:, h:h + 1], None, AX.add)

            # ---- phase 4: attention (S^T layout, staged exp, fused denom) ----
            with tc.tile_pool(name="oattp", bufs=1) as oattp, \
                 tc.tile_pool(name="epool", bufs=2) as epool, \
                 tc.tile_pool(name="psum4", bufs=1, space="PSUM") as psum4:
                OATT = [oattp.tile([DH, NQ], F32R, name=f"oatt{h}") for h in range(HEADS)]
                QP = [(0, 1024, [(0, 512), (512, 512)]),
                      (1024, 348, [(0, 348)])]
                for h in range(HEADS):
                    for qo, qw, subs in QP:
                        pavs = [psum4.tile([65, sz], F32, name=f"pav{si}",
                                           tag=f"pav{si}", bufs=1)
                                for si, (so, sz) in enumerate(subs)]
                        for t, (mo, msz) in enumerate(MT22):
                            ps = psum4.tile([128, qw], F32, name="ps", tag="ps", bufs=2,
                                            padded_shape=[128, 1024])
                            for so, sz in subs:
                                nc.tensor.matmul(ps[0:msz, so:so + sz],
                                                 KHAT[h][:, mo:mo + msz],
                                                 QHAT[h][:, qo + so:qo + so + sz],
                                                 start=True, stop=True)
                            e = epool.tile([128, qw], F32R, name="e", tag="e", bufs=6)
                            nc.scalar.activation(e[0:msz, :], ps[0:msz, :], AF.Exp)
                            for si, (so, sz) in enumerate(subs):
                                nc.tensor.matmul(pavs[si][:],
                                                 VTON[h][0:msz, 65 * t:65 * t + 65],
                                                 e[0:msz, so:so + sz],
                                                 start=(t == 0), stop=(t == 21))
                        for si, (so, sz) in enumerate(subs):
                            pav = pavs[si]
                            off = qo + so
                            rcr = epool.tile([65, sz], F32R, name="rcr", tag="rcr", bufs=2)
                            with nc.allow_low_precision(reason="softmax denom recip"):
                                nc.vector.reciprocal(rcr[DH:65, :], pav[DH:65, :])
                            pb = psum4.tile([DH, sz], F32, name="pb", tag="pb", bufs=1)
                            nc.tensor.matmul(pb[:], onesr[DH:65, :], rcr[DH:65, :],
                                             start=True, stop=True)
                            pbs = epool.tile([DH, sz], F32, name="pbs", tag="pbs", bufs=2)
                            nc.vector.tensor_copy(pbs[:], pb[:])
                            nc.vector.tensor_tensor(OATT[h][:, off:off + sz],
                                                    pav[0:DH, :], pbs[:], AX.mult)
                            idx = h * 4 + (0 if qo == 0 else 2) + si
                            nc.vector.reduce_sum(S2[:, idx:idx + 1],
                                                 OATT[h][:, off:off + sz], X_AXIS)
                            sq2 = epool.tile([DH, sz], F32, name="sq2", tag="sq2", bufs=2)
                            nc.vector.tensor_tensor(sq2[:], OATT[h][:, off:off + sz],
                                                    OATT[h][:, off:off + sz], AX.mult)
                            nc.vector.reduce_sum(Q2[:, idx:idx + 1], sq2[:], X_AXIS)

                # ---- phase 5: BN2 + relu (DVE), assemble OUT2 ----
                s2sum = wpool.tile([DH, 4], F32, name="s2sum")
                q2sum = wpool.tile([DH, 4], F32, name="q2sum")
                nc.vector.reduce_sum(s2sum[:], S2[:].rearrange("p (h c) -> p h c", c=4), X_AXIS)
                nc.vector.reduce_sum(q2sum[:], Q2[:].rearrange("p (h c) -> p h c", c=4), X_AXIS)
                st2 = wpool.tile([DH, 8], F32, name="st2")
                nc.vector.tensor_copy(st2[:, 0:4], s2sum[:])
                nc.vector.tensor_copy(st2[:, 4:8], q2sum[:])
                tot2 = wpool.tile([DH, 8], F32, name="tot2")
                stats_collective(st2, 8, tot2, "bn2")
                s2c = wpool.tile([DH, 4], F32, name="s2c")
                c2c = wpool.tile([DH, 4], F32, name="c2c")
                bn_coeffs(tot2, g2t, b2t, CNT2, 4, s2c, c2c, "bn2")
                for h in range(HEADS):
                    nc.vector.tensor_scalar(OATT[h][:], OATT[h][:],
                                            s2c[:, h:h + 1], c2c[:, h:h + 1],
                                            AX.mult, AX.add)
                    nc.vector.tensor_scalar(OATT[h][:], OATT[h][:], 0.0, None, AX.max)
                    nc.scalar.dma_start(
                        OUT2[h // 2][(h % 2) * DH:(h % 2) * DH + DH, :], OATT[h][:])

        # ---- phase 6: y3 = W3 @ out2, stats; finale ----
        with tc.tile_pool(name="y3pool", bufs=1) as y3pool, \
             tc.tile_pool(name="fpool", bufs=2) as fpool, \
             tc.tile_pool(name="psum6", bufs=3, space="PSUM") as psum6:
            Y3 = [y3pool.tile([128, NQ], F32, name=f"y3_{m}") for m in range(8)]
            XR = []
            for mt in range(8):
                xr = fpool.tile([128, NQ], F32, name=f"xr{mt}", tag=f"xr{mt}", bufs=1)
                nc.scalar.dma_start(xr[:], Xd[mt * 128:(mt + 1) * 128, 0:NQ].bitcast(F32))
                XR.append(xr)
            for mt in range(8):
                for ci, (off, sz) in enumerate(CH3):
                    ps = psum6.tile([128, sz], F32, name="pw3", tag="pw3")
                    for k in range(2):
                        nc.tensor.matmul(ps[:], w3t[k][:, mt * 128:(mt + 1) * 128],
                                         OUT2[k][:, off:off + sz],
                                         start=(k == 0), stop=(k == 1))
                    idx = mt * 3 + ci
                    nc.scalar.copy(Y3[mt][:, off:off + sz], ps[:])
                    nc.vector.reduce_sum(S3[:, idx:idx + 1], ps[:], X_AXIS)
                    sq = fpool.tile([128, sz], F32, name="sq3", tag="sq3")
                    nc.vector.tensor_tensor(sq[:], Y3[mt][:, off:off + sz],
                                            Y3[mt][:, off:off + sz], AX.mult)
                    nc.vector.reduce_sum(Q3[:, idx:idx + 1], sq[:], X_AXIS)

            s3sum = wpool.tile([128, 8], F32, name="s3sum")
            q3sum = wpool.tile([128, 8], F32, name="q3sum")
            nc.vector.reduce_sum(s3sum[:], S3[:].rearrange("p (m c) -> p m c", c=3), X_AXIS)
            nc.vector.reduce_sum(q3sum[:], Q3[:].rearrange("p (m c) -> p m c", c=3), X_AXIS)
            st3 = wpool.tile([128, 16], F32, name="st3")
            nc.vector.tensor_copy(st3[:, 0:8], s3sum[:])
            nc.vector.tensor_copy(st3[:, 8:16], q3sum[:])
            tot3 = wpool.tile([128, 16], F32, name="tot3")
            stats_collective(st3, 16, tot3, "bn3")
            s3c = wpool.tile([128, 8], F32, name="s3c")
            c3c = wpool.tile([128, 8], F32, name="c3c")
            bn_coeffs(tot3, g3t, b3t, CNT2, 8, s3c, c3c, "bn3")

            for mt in range(8):
                xr = XR[mt]
                tf = fpool.tile([128, NQ], F32, name="tf", tag="tf")
                nc.vector.scalar_tensor_tensor(tf[:], Y3[mt][:], s3c[:, mt:mt + 1],
                                               xr[:], AX.mult, AX.add)
                to = fpool.tile([128, NQ], F32, name="to", tag="to")
                nc.vector.tensor_scalar(to[:], tf[:], c3c[:, mt:mt + 1], 0.0,
                                        AX.add, AX.max)
                nc.sync.dma_start(OUTd[mt * 128:(mt + 1) * 128, :], to[:])


_NC_CACHE = {}


def _get_program():
    if "nc" not in _NC_CACHE:
        _NC_CACHE["nc"] = build_program()
    return _NC_CACHE["nc"]


def _host_prep(inputs):
    x = np.ascontiguousarray(inputs["x"].reshape(B, CIN, N))
    rel = (inputs["rel_h"] + inputs["rel_w"] + inputs["rel_d"]).reshape(HEADS, DH, N)
    rel = np.ascontiguousarray(rel.astype(np.float32))
    W1T = np.ascontiguousarray(inputs["W1"].T.astype(np.float32))
    WQT = np.ascontiguousarray(inputs["Wq"].T.astype(np.float32))
    WKT = np.ascontiguousarray(inputs["Wk"].T.astype(np.float32))
    WVT = np.ascontiguousarray(inputs["Wv"].T.astype(np.float32))
    W3T = np.ascontiguousarray(inputs["W3"].T.astype(np.float32))
    WKQ = np.stack([np.concatenate([WKT[:, h * DH:(h + 1) * DH],
                                    WQT[:, h * DH:(h + 1) * DH]], axis=1)
                    for h in range(HEADS)]).astype(np.float32)
    bq, bk, bv = inputs["bq"], inputs["bk"], inputs["bv"]
    BKQ = np.stack([np.concatenate([bk[h * DH:(h + 1) * DH], bq[h * DH:(h + 1) * DH]])
                    for h in range(HEADS)]).astype(np.float32)
    BQ = bq.reshape(HEADS, DH).astype(np.float32)
    BVR = bv.reshape(1, C).astype(np.float32)
    GB1 = np.stack([inputs["g1"], inputs["b1"]]).astype(np.float32)
    GB2 = np.stack([inputs["g2"], inputs["b2"]]).astype(np.float32)
    GB3 = np.stack([inputs["g3"], inputs["b3"]]).astype(np.float32)

    in_maps = []
    for c in range(N_CORES):
        b, s = c // 2, c % 2
        xb = np.roll(x[b], -s * NQ, axis=1)
        relc = np.ascontiguousarray(rel[:, :, s * NQ:(s + 1) * NQ])
        in_maps.append({
            "X": np.ascontiguousarray(xb), "W1T": W1T, "WQT": WQT,
            "WVT": WVT, "W3T": W3T, "WKQ": WKQ, "REL": relc, "BKQ": BKQ,
            "BQ": BQ, "BVR": BVR, "GB1": GB1, "GB2": GB2, "GB3": GB3,
        })
    return in_maps


def run(inputs, trace=False, trace_kwargs=None):
    from concourse import bass_utils
    nc = _get_program()
    in_maps = _host_prep(inputs)
    res = bass_utils.run_bass_kernel_spmd(
        nc, in_maps, core_ids=list(range(N_CORES)), trace=trace,
        **(trace_kwargs or {}))
    out = np.empty((B, CIN, N), np.float32)
    for c in range(N_CORES):
        b, s = c // 2, c % 2
        out[b, :, s * NQ:(s + 1) * NQ] = res.results[c]["OUT"]
    return out.reshape(B, CIN, 14, 14, 14), res


def kernel(**inputs):
    out, _ = run(inputs, trace=False)
    return out

